# revision 10
# baseline (speedup 1.0000x reference)
"""Distributed Trainium2 kernel for nn_Attention (B=2, N=2048, C=1024, H=16, HD=64).

Sharding: sequence-parallel. Core c owns batch b=c//4 and query rows
[512*(c%4), 512*(c%4+1)).  Each core computes q/k/v for its own rows,
RoPEs q and k, AllGathers k^T and v (within its 4-core batch group),
then computes attention + projection for its row slice.  Outputs are
disjoint row slices of the final [B, N, C] tensor — no reduction needed.

All matmuls run in float32r (full-rate fp32).  Weights are pre-transposed
on the host so every matmul operand has its natural layout on device.
"""

import sys

if "/opt/trn_rl_repo" not in sys.path:
    sys.path.insert(0, "/opt/trn_rl_repo")

import numpy as np

B, N, C = 2, 2048, 1024
H, HD = 16, 64
NCORES = 8
GB = 4          # cores per batch (replica group size)
NS = N // GB    # 512 rows per core
SC = HD ** -0.5  # attention scale


def build():
    import concourse.bass as bass
    import concourse.mybir as mybir
    import concourse.tile as tile
    from concourse import bacc

    f32 = mybir.dt.float32
    f32r = mybir.dt.float32r
    AF = mybir.ActivationFunctionType

    nc = bacc.Bacc(None, target_bir_lowering=False, num_devices=NCORES)

    # ---- per-core external inputs (host pre-shards / pre-transposes) ----
    xT = nc.declare_dram_parameter("xT", [C, NS], f32r, isOutput=False)        # x[b, rows].T
    wqkT = nc.declare_dram_parameter("wqkT", [C, 2 * C], f32r, isOutput=False)  # qkv_w[:2C].T
    wvT = nc.declare_dram_parameter("wvT", [C, C], f32r, isOutput=False)       # qkv_w[2C:].T
    wpT = nc.declare_dram_parameter("wpT", [C, C], f32r, isOutput=False)       # proj_w.T
    cos2 = nc.declare_dram_parameter("cos2", [128, NS], f32, isOutput=False)  # cosT doubled
    sins2 = nc.declare_dram_parameter("sins2", [128, NS], f32, isOutput=False)  # signed sinT doubled
    biasb = nc.declare_dram_parameter("biasb", [128, C], f32, isOutput=False)  # proj_b bcast
    out = nc.declare_dram_parameter("out", [NS, C], f32, isOutput=True)

    groups = [list(range(GB)), list(range(GB, 2 * GB))]

    def mm(out_ap, lhsT_ap, rhs_ap, start, stop):
        nc.tensor.matmul(out_ap, lhsT_ap, rhs_ap, start=start, stop=stop)

    from contextlib import ExitStack

    with tile.TileContext(nc) as tc:
        with ExitStack() as stack:
            ep = stack.enter_context
            ep(nc.allow_low_precision(reason="f32r rounding of fp32 matmul inputs"))
            dramp = ep(tc.tile_pool(name="dram", bufs=1, space="DRAM"))
            constp = ep(tc.tile_pool(name="const", bufs=1))
            xtp = ep(tc.tile_pool(name="xTp", bufs=1))
            qtp = ep(tc.tile_pool(name="qTp", bufs=1))
            atp = ep(tc.tile_pool(name="aTp", bufs=1))
            wqkp = ep(tc.tile_pool(name="wqk", bufs=6))
            wvp = ep(tc.tile_pool(name="wv", bufs=16))
            wpp = ep(tc.tile_pool(name="wp", bufs=16))
            ktmpp = ep(tc.tile_pool(name="ktmp", bufs=3))
            ropep = ep(tc.tile_pool(name="ropet", bufs=3))
            kheadp = ep(tc.tile_pool(name="khead", bufs=2))
            ptp = ep(tc.tile_pool(name="pTp", bufs=4))
            vtp = ep(tc.tile_pool(name="vtile", bufs=4))
            smallp = ep(tc.tile_pool(name="small", bufs=4))
            outp = ep(tc.tile_pool(name="outsb", bufs=3))
            ps_mm = ep(tc.tile_pool(name="ps_mm", bufs=2, space="PSUM"))
            ps_s = ep(tc.tile_pool(name="ps_s", bufs=2, space="PSUM"))
            ps_av = ep(tc.tile_pool(name="ps_av", bufs=2, space="PSUM"))
            ps_bc = ep(tc.tile_pool(name="ps_bc", bufs=2, space="PSUM"))
            # ---- internal DRAM for collectives ----
            k_in = dramp.tile([C, NS], f32r, name="k_in")
            k_gath = dramp.tile([GB, C, NS], f32r, name="k_gath")
            v_in = dramp.tile([NS, H, HD + 1], f32r, name="v_in")
            v_gath = dramp.tile([GB, NS, H, HD + 1], f32r, name="v_gath")

            # ---- constants / persistent loads ----
            cos_sb = constp.tile([128, NS], f32, name="cos_sb")
            nc.sync.dma_start(cos_sb[:, :], cos2[:, :])
            sin_sb = constp.tile([128, NS], f32, name="sin_sb")
            nc.sync.dma_start(sin_sb[:, :], sins2[:, :])
            bias_sb = constp.tile([128, C], f32, name="bias_sb")
            nc.sync.dma_start(bias_sb[:, :], biasb[:, :])
            onesf = constp.tile([128, 64], f32, name="onesf")
            nc.vector.memset(onesf[:, :], 1.0)
            ones64 = constp.tile([1, 64], f32r, name="ones64")
            nc.vector.tensor_copy(ones64[:, :], onesf[0:1, :])
            ones16 = constp.tile([128, H], f32r, name="ones16")
            nc.vector.tensor_copy(ones16[:, :], onesf[:, 0:H])

            xT_sb = xtp.tile([128, 8, NS], f32r, name="xT_sb")
            for cc in range(8):
                nc.sync.dma_start(xT_sb[:, cc, :], xT[cc * 128:(cc + 1) * 128, :])

            qT_sb = qtp.tile([128, 8, NS], f32r, name="qT_sb")
            aT_sb = atp.tile([128, 8, NS], f32r, name="aT_sb")

            def rope_chunk(psum, dst):
                """dst = psum*cos + rot32(psum)*signed_sin, all [128, NS]."""
                tmp = ropep.tile([128, NS], f32, name="tmp", tag="ropetmp")
                # rot: swap 32-row halves within each 64-row head
                for lo in (0, 64):
                    nc.vector.tensor_mul(
                        tmp[lo:lo + 32, :],
                        psum[lo + 32:lo + 64, :],
                        sin_sb[lo:lo + 32, :],
                    )
                    nc.vector.tensor_mul(
                        tmp[lo + 32:lo + 64, :],
                        psum[lo:lo + 32, :],
                        sin_sb[lo + 32:lo + 64, :],
                    )
                nc.vector.tensor_mul(dst, psum, cos_sb[:, :])
                nc.vector.tensor_add(dst, dst, tmp[:, :])

            # ---- k^T group (dq chunks 8..15) -> rope -> k_in ----
            for m in range(8, 16):
                psum = ps_mm.tile([128, NS], f32, name="psum", tag="mm")
                for cc in range(8):
                    w = wqkp.tile([128, 128], f32r, name="w", tag="wqk")
                    nc.sync.dma_start(
                        w[:, :], wqkT[cc * 128:(cc + 1) * 128, m * 128:(m + 1) * 128]
                    )
                    mm(psum[:, :], w[:, :], xT_sb[:, cc, :], cc == 0, cc == 7)
                kc = ktmpp.tile([128, NS], f32r, name="kc", tag="kc")
                rope_chunk(psum[:, :], kc[:, :])
                nc.sync.dma_start(k_in[(m - 8) * 128:(m - 7) * 128, :], kc[:, :])

            # ---- v natural layout [i, dv] -> v_in (with ones column) ----
            wv_tiles = {}
            for nn in range(2):
                for cc in range(8):
                    w = wvp.tile([128, 512], f32r, name="w", tag="wv")
                    nc.sync.dma_start(
                        w[:, :], wvT[cc * 128:(cc + 1) * 128, nn * 512:(nn + 1) * 512]
                    )
                    wv_tiles[(nn, cc)] = w
            for ic in range(4):
                rows = slice(ic * 128, (ic + 1) * 128)
                for nn in range(2):
                    psum = ps_mm.tile([128, NS], f32, name="psum", tag="mm")
                    for cc in range(8):
                        mm(
                            psum[:, :],
                            xT_sb[:, cc, rows],
                            wv_tiles[(nn, cc)][:, :],
                            cc == 0,
                            cc == 7,
                        )
                    vsb = outp.tile([128, 512], f32r, name="vsb", tag="vsb")
                    nc.vector.tensor_copy(vsb[:, :], psum[:, :])
                    nc.sync.dma_start(
                        v_in[rows, nn * 8:(nn + 1) * 8, 0:HD],
                        vsb[:, :].rearrange("p (h d) -> p h d", h=8),
                    )
                nc.sync.dma_start(v_in[rows, :, HD], ones16[:, :])

            nc.gpsimd.collective_compute(
                "AllGather",
                mybir.AluOpType.bypass,
                replica_groups=groups,
                ins=[k_in.opt()],
                outs=[k_gath.opt()],
            )
            nc.gpsimd.collective_compute(
                "AllGather",
                mybir.AluOpType.bypass,
                replica_groups=groups,
                ins=[v_in.opt()],
                outs=[v_gath.opt()],
            )

            # ---- q^T group (dq chunks 0..7) + rope, overlaps the gathers ----
            for m in range(8):
                psum = ps_mm.tile([128, NS], f32, name="psum", tag="mm")
                for cc in range(8):
                    w = wqkp.tile([128, 128], f32r, name="w", tag="wqk")
                    nc.sync.dma_start(
                        w[:, :], wqkT[cc * 128:(cc + 1) * 128, m * 128:(m + 1) * 128]
                    )
                    mm(psum[:, :], w[:, :], xT_sb[:, cc, :], cc == 0, cc == 7)
                rope_chunk(psum[:, :], qT_sb[:, m, :])

            # ---- attention, head pairs (flash-style over key chunks) ----
            for hp in range(H // 2):  # heads 2*hp, 2*hp+1
                kh = kheadp.tile([128, GB, NS], f32r, name="kh", tag="khead")
                nc.sync.dma_start(
                    kh[:, :, :],
                    k_gath[:, hp * 128:(hp + 1) * 128, :].transpose([1, 0, 2]),
                )
                for sub in range(2):  # head h = 2*hp + sub at partitions sub*64
                    h = 2 * hp + sub
                    lo = sub * 64
                    q_ap = qT_sb[lo:lo + 64, hp, :]
                    po = ps_av.tile([HD + 1, NS], f32, name="po", tag="av")
                    for jc in range(16):
                        r, jl = jc // 4, jc % 4
                        ps = ps_s.tile([128, NS], f32, name="ps", tag="sc")
                        mm(
                            ps[:, :],
                            kh[lo:lo + 64, r, jl * 128:(jl + 1) * 128],
                            q_ap,
                            True,
                            True,
                        )
                        pt = ptp.tile([128, NS], f32r, name="pt", tag="pT")
                        nc.scalar.activation(pt[:, :], ps[:, :], AF.Exp, scale=SC)
                        vt = vtp.tile([128, HD + 1], f32r, name="vt", tag="vt")
                        nc.sync.dma_start(
                            vt[:, :], v_gath[r, jl * 128:(jl + 1) * 128, h, :]
                        )
                        mm(po[:, :], vt[:, :], pt[:, :], jc == 0, jc == 15)
                    # normalize: reciprocal of denom row, bcast via K=1 matmul
                    recip = smallp.tile([1, NS], f32r, name="recip", tag="recip")
                    nc.vector.reciprocal(recip[:, :], po[HD:HD + 1, :])
                    pb = ps_bc.tile([64, NS], f32, name="pb", tag="bc")
                    mm(pb[:, :], ones64[:, :], recip[:, :], True, True)
                    rb = smallp.tile([64, NS], f32, name="rb", tag="rb")
                    nc.scalar.copy(rb[:, :], pb[:, :])
                    nc.vector.tensor_mul(
                        aT_sb[lo:lo + 64, hp, :],
                        po[0:HD, :],
                        rb[:, :],
                    )

            # ---- projection ----
            wp_tiles = {}
            for nn in range(2):
                for cc in range(8):
                    w = wpp.tile([128, 512], f32r, name="w", tag="wp")
                    nc.sync.dma_start(
                        w[:, :], wpT[cc * 128:(cc + 1) * 128, nn * 512:(nn + 1) * 512]
                    )
                    wp_tiles[(nn, cc)] = w
            for ic in range(4):
                rows = slice(ic * 128, (ic + 1) * 128)
                for nn in range(2):
                    psum = ps_mm.tile([128, NS], f32, name="psum", tag="mm")
                    for cc in range(8):
                        mm(
                            psum[:, :],
                            aT_sb[:, cc, rows],
                            wp_tiles[(nn, cc)][:, :],
                            cc == 0,
                            cc == 7,
                        )
                    osb = outp.tile([128, 512], f32, name="osb", tag="osb")
                    nc.vector.tensor_add(
                        osb[:, :], psum[:, :], bias_sb[:, nn * 512:(nn + 1) * 512]
                    )
                    nc.sync.dma_start(out[rows, nn * 512:(nn + 1) * 512], osb[:, :])

    nc.compile()
    return nc


_NC_CACHE = {}


def _get_nc():
    if "nc" not in _NC_CACHE:
        _NC_CACHE["nc"] = build()
    return _NC_CACHE["nc"]


def make_in_maps(x, cos, sin, qkv_w, proj_w, proj_b):
    x = np.asarray(x, np.float32)
    cos = np.asarray(cos, np.float32)
    sin = np.asarray(sin, np.float32)
    qkv_w = np.asarray(qkv_w, np.float32)
    proj_w = np.asarray(proj_w, np.float32)
    proj_b = np.asarray(proj_b, np.float32)

    wqkT = np.ascontiguousarray(qkv_w[: 2 * C].T)        # [C, 2C]
    wvT = np.ascontiguousarray(qkv_w[2 * C:].T)          # [C, C]
    wpT = np.ascontiguousarray(proj_w.T)                 # [C, C]
    biasb = np.ascontiguousarray(np.broadcast_to(proj_b, (128, C)))
    sign = np.concatenate([-np.ones(32, np.float32), np.ones(32, np.float32)])

    in_maps = []
    for c in range(NCORES):
        b, r = c // GB, c % GB
        rows = slice(r * NS, (r + 1) * NS)
        xTc = np.ascontiguousarray(x[b, rows].T)          # [C, NS]
        cosT = cos[rows].T                                # [HD, NS]
        sinsT = (sin[rows] * sign).T                      # [HD, NS] signed
        cos2v = np.ascontiguousarray(np.concatenate([cosT, cosT], 0))   # [128, NS]
        sins2v = np.ascontiguousarray(np.concatenate([sinsT, sinsT], 0))
        in_maps.append(
            {
                "xT": xTc,
                "wqkT": wqkT,
                "wvT": wvT,
                "wpT": wpT,
                "cos2": cos2v,
                "sins2": sins2v,
                "biasb": biasb,
            }
        )
    return in_maps


def assemble(results):
    out = np.empty((B, N, C), np.float32)
    for c in range(NCORES):
        b, r = c // GB, c % GB
        out[b, r * NS:(r + 1) * NS] = results[c]["out"]
    return out


def kernel(x, cos, sin, qkv_w, proj_w, proj_b):
    from concourse.bass_utils import run_bass_kernel_spmd

    nc = _get_nc()
    in_maps = make_in_maps(x, cos, sin, qkv_w, proj_w, proj_b)
    res = run_bass_kernel_spmd(nc, in_maps, core_ids=list(range(NCORES)))
    return assemble(res.results)


# revision 25
# speedup vs baseline: 1.0108x; 1.0108x over previous
"""Distributed Trainium2 kernel for nn_Attention (B=2, N=2048, C=1024, H=16, HD=64).

Sharding: sequence-parallel. Core c owns batch b=c//4 and query rows
[512*(c%4), 512*(c%4+1)).  Each core computes q/k/v for its own rows,
RoPEs q and k, AllGathers k^T and v (within its 4-core batch group),
then computes attention + projection for its row slice.  Outputs are
disjoint row slices of the final [B, N, C] tensor — no reduction needed.

All matmuls run in float32r (full-rate fp32).  Weights are pre-transposed
on the host so every matmul operand has its natural layout on device.
Attention is computed transposed (S^T = k^T q) so softmax denominators
come from an appended ones-column in v, and no on-device transposes are
ever needed.
"""

import sys

if "/opt/trn_rl_repo" not in sys.path:
    sys.path.insert(0, "/opt/trn_rl_repo")

import numpy as np

B, N, C = 2, 2048, 1024
H, HD = 16, 64
NCORES = 8
GB = 4          # cores per batch (replica group size)
NS = N // GB    # 512 rows per core
SC = HD ** -0.5  # attention scale


def build(mock_ag=False):
    import concourse.bass as bass
    import concourse.mybir as mybir
    import concourse.tile as tile
    from concourse import bacc

    f32 = mybir.dt.float32
    f32r = mybir.dt.float32r
    AF = mybir.ActivationFunctionType

    nc = bacc.Bacc(None, target_bir_lowering=False, num_devices=NCORES)

    # ---- per-core external inputs (host pre-shards / pre-transposes) ----
    xT = nc.declare_dram_parameter("xT", [C, NS], f32r, isOutput=False)
    wqkT = nc.declare_dram_parameter("wqkT", [C, 2 * C], f32r, isOutput=False)
    wvT = nc.declare_dram_parameter("wvT", [C, C], f32r, isOutput=False)
    wpT = nc.declare_dram_parameter("wpT", [C, C], f32r, isOutput=False)
    cos2 = nc.declare_dram_parameter("cos2", [128, NS], f32, isOutput=False)
    sins2 = nc.declare_dram_parameter("sins2", [128, NS], f32, isOutput=False)
    biasb = nc.declare_dram_parameter("biasb", [128, C], f32, isOutput=False)
    out = nc.declare_dram_parameter("out", [NS, C], f32, isOutput=True)

    groups = [list(range(GB)), list(range(GB, 2 * GB))]

    def mm(out_ap, lhsT_ap, rhs_ap, start, stop):
        nc.tensor.matmul(out_ap, lhsT_ap, rhs_ap, start=start, stop=stop)

    from contextlib import ExitStack

    with tile.TileContext(nc) as tc:
        with ExitStack() as stack:
            ep = stack.enter_context
            ep(nc.allow_low_precision(reason="f32r rounding of fp32 matmul inputs"))
            dramp = ep(tc.tile_pool(name="dram", bufs=1, space="DRAM"))
            constp = ep(tc.tile_pool(name="const", bufs=1))
            xtp = ep(tc.tile_pool(name="xTp", bufs=1))
            qtp = ep(tc.tile_pool(name="qTp", bufs=1))
            atp = ep(tc.tile_pool(name="aTp", bufs=1))
            wtsp = ep(tc.tile_pool(name="wts", bufs=12))
            ktmpp = ep(tc.tile_pool(name="ktmp", bufs=3))
            ropep = ep(tc.tile_pool(name="ropet", bufs=3))
            kheadp = ep(tc.tile_pool(name="khead", bufs=2))
            ptp = ep(tc.tile_pool(name="pTp", bufs=3))
            vhp_p = ep(tc.tile_pool(name="vhp", bufs=4))
            smallp = ep(tc.tile_pool(name="small", bufs=4))
            outp = ep(tc.tile_pool(name="outsb", bufs=3))
            ps_mm = ep(tc.tile_pool(name="ps_mm", bufs=2, space="PSUM"))
            ps_s = ep(tc.tile_pool(name="ps_s", bufs=2, space="PSUM"))
            ps_av = ep(tc.tile_pool(name="ps_av", bufs=2, space="PSUM"))

            # ---- internal DRAM for collectives ----
            k_in = dramp.tile([C, NS], f32r, name="k_in")
            k_gath = dramp.tile([GB, C, NS], f32r, name="k_gath")
            v_in = dramp.tile([NS, H, HD + 1], f32r, name="v_in")
            v_gath = dramp.tile([GB, NS, H, HD + 1], f32r, name="v_gath")

            # ---- constants / persistent loads ----
            cos_sb = constp.tile([128, NS], f32, name="cos_sb")
            nc.sync.dma_start(cos_sb[:, :], cos2[:, :])
            sin_sb = constp.tile([128, NS], f32, name="sin_sb")
            nc.sync.dma_start(sin_sb[:, :], sins2[:, :])
            bias_sb = constp.tile([128, C], f32, name="bias_sb")
            nc.sync.dma_start(bias_sb[:, :], biasb[:, :])
            onesf = constp.tile([128, 64], f32, name="onesf")
            nc.vector.memset(onesf[:, :], 1.0)
            ones64 = constp.tile([1, 64], f32r, name="ones64")
            nc.vector.tensor_copy(ones64[:, :], onesf[0:1, :])

            xT_sb = xtp.tile([128, 8, NS], f32r, name="xT_sb")
            for cc in range(8):
                nc.sync.dma_start(
                    xT_sb[:, cc, :], xT[cc * 128:(cc + 1) * 128, :]
                )

            qT_sb = qtp.tile([128, 8, NS], f32r, name="qT_sb")
            aT_sb = atp.tile([128, 8, NS], f32r, name="aT_sb")

            def rope_chunk(psum, dst):
                """dst = psum*cos + rot32(psum)*signed_sin, all [128, NS]."""
                tmp = ropep.tile([128, NS], f32, name="tmp", tag="ropetmp")
                for lo in (0, 64):
                    nc.vector.tensor_mul(
                        tmp[lo:lo + 32, :],
                        psum[lo + 32:lo + 64, :],
                        sin_sb[lo:lo + 32, :],
                    )
                    nc.vector.tensor_mul(
                        tmp[lo + 32:lo + 64, :],
                        psum[lo:lo + 32, :],
                        sin_sb[lo + 32:lo + 64, :],
                    )
                nc.vector.tensor_mul(dst, psum, cos_sb[:, :])
                nc.vector.tensor_add(dst, dst, tmp[:, :])

            # ---- k^T group (dq chunks 8..15) -> rope -> k_in -> AllGather ----
            wk_tiles = []
            for cc in range(8):
                w = wtsp.tile([128, 8, 128], f32r, name="w", tag="wts")
                for half in range(2):
                    nc.sync.dma_start(
                        w[:, half * 4:(half + 1) * 4, :],
                        wqkT[
                            cc * 128:(cc + 1) * 128,
                            C + half * 512:C + (half + 1) * 512,
                        ].rearrange("p (m f) -> p m f", f=128),
                    )
                wk_tiles.append(w)
            for m in range(8):
                psum = ps_mm.tile([128, NS], f32, name="psum", tag="mm")
                for cc in range(8):
                    mm(psum[:, :], wk_tiles[cc][:, m, :], xT_sb[:, cc, :],
                       cc == 0, cc == 7)
                kc = ktmpp.tile([128, NS], f32r, name="kc", tag="kc")
                rope_chunk(psum[:, :], kc[:, :])
                nc.sync.dma_start(k_in[m * 128:(m + 1) * 128, :], kc[:, :])

            if mock_ag:
                # timing stand-in: ~real AllGather latency, correct dep edges
                for r in range(GB):
                    nc.gpsimd.dma_start(
                        k_gath[r, 0:64, :], k_in[0:64, :]
                    )
            else:
                nc.gpsimd.collective_compute(
                    "AllGather",
                    mybir.AluOpType.bypass,
                    replica_groups=groups,
                    ins=[k_in.opt()],
                    outs=[k_gath.opt()],
                )

            # ---- v natural layout [i, dv] -> v_in (ones col) -> AllGather ----
            wv_tiles = []
            for cc in range(8):
                w = wtsp.tile([128, 2, 512], f32r, name="w", tag="wts")
                nc.sync.dma_start(
                    w[:, :, :],
                    wvT[cc * 128:(cc + 1) * 128, :].rearrange(
                        "p (n f) -> p n f", f=512
                    ),
                )
                wv_tiles.append(w)
            for ic in range(4):
                rows = slice(ic * 128, (ic + 1) * 128)
                vsb = outp.tile([128, H, HD + 1], f32r, name="vsb", tag="osb")
                nc.vector.tensor_copy(vsb[:, :, HD], onesf[:, 0:H])
                for nn in range(2):
                    psum = ps_mm.tile([128, NS], f32, name="psum", tag="mm")
                    for cc in range(8):
                        mm(psum[:, :], xT_sb[:, cc, rows], wv_tiles[cc][:, nn, :],
                           cc == 0, cc == 7)
                    nc.vector.tensor_copy(
                        vsb[:, nn * 8:(nn + 1) * 8, 0:HD],
                        psum[:, :].rearrange("p (h d) -> p h d", d=HD),
                    )
                nc.sync.dma_start(v_in[rows, :, :], vsb[:, :, :])
            if mock_ag:
                for r in range(GB):
                    nc.gpsimd.dma_start(
                        v_gath[r, 0:32, :, :], v_in[0:32, :, :]
                    )
            else:
                nc.gpsimd.collective_compute(
                    "AllGather",
                    mybir.AluOpType.bypass,
                    replica_groups=groups,
                    ins=[v_in.opt()],
                    outs=[v_gath.opt()],
                )

            # ---- q^T group (dq chunks 0..7) + rope, overlaps the gathers ----
            wq_tiles = []
            for cc in range(8):
                w = wtsp.tile([128, 8, 128], f32r, name="w", tag="wts")
                nc.sync.dma_start(
                    w[:, :, :],
                    wqkT[cc * 128:(cc + 1) * 128, 0:C].rearrange(
                        "p (m f) -> p m f", f=128
                    ),
                )
                wq_tiles.append(w)
            for m in range(8):
                psum = ps_mm.tile([128, NS], f32, name="psum", tag="mm")
                for cc in range(8):
                    mm(psum[:, :], wq_tiles[cc][:, m, :], xT_sb[:, cc, :],
                       cc == 0, cc == 7)
                rope_chunk(psum[:, :], qT_sb[:, m, :])

            # ---- attention, head pairs (flash-style over key chunks) ----
            vg = {}
            for hp in range(H // 2):  # heads 2*hp, 2*hp+1
                if hp % 4 == 0:  # prefetch v for heads [8*g, 8*(g+1))
                    g = hp // 4
                    for r in range(GB):
                        vt = vhp_p.tile(
                            [128, GB, 8, HD + 1], f32r, name="vt", tag="vt"
                        )
                        nc.gpsimd.dma_start(
                            vt[:, :, :, :],
                            v_gath[r, :, g * 8:(g + 1) * 8, :].rearrange(
                                "(a p) h d -> p a h d", p=128
                            ),
                        )
                        vg[r] = vt
                kh = kheadp.tile([128, GB, NS], f32r, name="kh", tag="khead")
                for r in range(GB):
                    nc.gpsimd.dma_start(
                        kh[:, r, :], k_gath[r, hp * 128:(hp + 1) * 128, :]
                    )
                for sub in range(2):  # head h = 2*hp + sub at partitions sub*64
                    h = 2 * hp + sub
                    lo = sub * 64
                    q_ap = qT_sb[lo:lo + 64, hp, :]
                    po = ps_av.tile([HD + 1, NS], f32, name="po", tag="av")
                    for jp in range(8):  # pairs of key chunks
                        jc0 = 2 * jp
                        ps2 = ps_s.tile([128, 2, NS], f32, name="ps2", tag="sc")
                        for u in range(2):
                            jc = jc0 + u
                            r, jl = jc // 4, jc % 4
                            mm(ps2[:, u, :],
                               kh[lo:lo + 64, r, jl * 128:(jl + 1) * 128],
                               q_ap, True, True)
                        pt = ptp.tile([128, 2, NS], f32r, name="pt", tag="pT")
                        nc.scalar.activation(
                            pt[:, :, :], ps2[:, :, :], AF.Exp, scale=SC
                        )
                        for u in range(2):
                            jc = jc0 + u
                            r, jl = jc // 4, jc % 4
                            mm(po[:, :],
                               vg[r][:, jl, 2 * (hp % 4) + sub, :],
                               pt[:, u, :], jc == 0, jc == 15)
                    # normalize: reciprocal of denom row, bcast via K=1 matmul
                    recip = smallp.tile([1, NS], f32r, name="recip", tag="recip")
                    nc.vector.reciprocal(recip[:, :], po[HD:HD + 1, :])
                    pb = ps_mm.tile([64, NS], f32, name="pb", tag="mm")
                    mm(pb[:, :], ones64[:, :], recip[:, :], True, True)
                    rb = smallp.tile([64, NS], f32, name="rb", tag="rb")
                    nc.vector.tensor_copy(rb[:, :], pb[:, :])
                    nc.vector.tensor_mul(
                        aT_sb[lo:lo + 64, hp, :], po[0:HD, :], rb[:, :]
                    )

            # ---- projection ----
            wp_tiles = []
            for cc in range(8):
                w = wtsp.tile([128, 2, 512], f32r, name="w", tag="wts")
                nc.sync.dma_start(
                    w[:, :, :],
                    wpT[cc * 128:(cc + 1) * 128, :].rearrange(
                        "p (n f) -> p n f", f=512
                    ),
                )
                wp_tiles.append(w)
            for ic in range(4):
                rows = slice(ic * 128, (ic + 1) * 128)
                for nn in range(2):
                    psum = ps_mm.tile([128, NS], f32, name="psum", tag="mm")
                    for cc in range(8):
                        mm(psum[:, :], aT_sb[:, cc, rows], wp_tiles[cc][:, nn, :],
                           cc == 0, cc == 7)
                    osb = outp.tile([128, 512], f32, name="osb", tag="osb")
                    nc.vector.tensor_add(
                        osb[:, :], psum[:, :], bias_sb[:, nn * 512:(nn + 1) * 512]
                    )
                    nc.sync.dma_start(out[rows, nn * 512:(nn + 1) * 512], osb[:, :])

    nc.compile()
    return nc


_NC_CACHE = {}


def _get_nc():
    if "nc" not in _NC_CACHE:
        _NC_CACHE["nc"] = build()
    return _NC_CACHE["nc"]


def make_in_maps(x, cos, sin, qkv_w, proj_w, proj_b):
    x = np.asarray(x, np.float32)
    cos = np.asarray(cos, np.float32)
    sin = np.asarray(sin, np.float32)
    qkv_w = np.asarray(qkv_w, np.float32)
    proj_w = np.asarray(proj_w, np.float32)
    proj_b = np.asarray(proj_b, np.float32)

    wqkT = np.ascontiguousarray(qkv_w[: 2 * C].T)        # [C, 2C]
    wvT = np.ascontiguousarray(qkv_w[2 * C:].T)          # [C, C]
    wpT = np.ascontiguousarray(proj_w.T)                 # [C, C]
    biasb = np.ascontiguousarray(np.broadcast_to(proj_b, (128, C)))
    sign = np.concatenate([-np.ones(32, np.float32), np.ones(32, np.float32)])

    in_maps = []
    for c in range(NCORES):
        b, r = c // GB, c % GB
        rows = slice(r * NS, (r + 1) * NS)
        xTc = np.ascontiguousarray(x[b, rows].T)          # [C, NS]
        cosT = cos[rows].T                                # [HD, NS]
        sinsT = (sin[rows] * sign).T                      # [HD, NS] signed
        cos2v = np.ascontiguousarray(np.concatenate([cosT, cosT], 0))   # [128, NS]
        sins2v = np.ascontiguousarray(np.concatenate([sinsT, sinsT], 0))
        in_maps.append(
            {
                "xT": xTc,
                "wqkT": wqkT,
                "wvT": wvT,
                "wpT": wpT,
                "cos2": cos2v,
                "sins2": sins2v,
                "biasb": biasb,
            }
        )
    return in_maps


def assemble(results):
    out = np.empty((B, N, C), np.float32)
    for c in range(NCORES):
        b, r = c // GB, c % GB
        out[b, r * NS:(r + 1) * NS] = results[c]["out"]
    return out


def kernel(x, cos, sin, qkv_w, proj_w, proj_b):
    from concourse.bass_utils import run_bass_kernel_spmd

    nc = _get_nc()
    in_maps = make_in_maps(x, cos, sin, qkv_w, proj_w, proj_b)
    res = run_bass_kernel_spmd(nc, in_maps, core_ids=list(range(NCORES)))
    return assemble(res.results)


# revision 28
# speedup vs baseline: 3738.4728x; 3698.5702x over previous
"""Distributed Trainium2 kernel for nn_Attention (B=2, N=2048, C=1024, H=16, HD=64).

Sharding: sequence-parallel. Core c owns batch b=c//4 and query rows
[512*(c%4), 512*(c%4+1)).  Each core computes q/k/v for its own rows,
RoPEs q and k, AllGathers k^T and v (within its 4-core batch group),
then computes attention + projection for its row slice.  Outputs are
disjoint row slices of the final [B, N, C] tensor — no reduction needed.

All matmuls run in float32r (full-rate fp32).  Weights are pre-transposed
on the host so every matmul operand has its natural layout on device.
Attention is computed transposed (S^T = k^T q) so softmax denominators
come from an appended ones-column in v, and no on-device transposes are
ever needed.
"""

import sys

if "/opt/trn_rl_repo" not in sys.path:
    sys.path.insert(0, "/opt/trn_rl_repo")

import numpy as np

B, N, C = 2, 2048, 1024
H, HD = 16, 64
NCORES = 8
GB = 4          # cores per batch (replica group size)
NS = N // GB    # 512 rows per core
SC = HD ** -0.5  # attention scale


def build(mock_ag=False):
    import concourse.bass as bass
    import concourse.mybir as mybir
    import concourse.tile as tile
    from concourse import bacc

    f32 = mybir.dt.float32
    f32r = mybir.dt.float32r
    AF = mybir.ActivationFunctionType

    nc = bacc.Bacc(None, target_bir_lowering=False, num_devices=NCORES)

    # ---- per-core external inputs (host pre-shards / pre-transposes) ----
    xT = nc.declare_dram_parameter("xT", [C, NS], f32r, isOutput=False)
    wqkT = nc.declare_dram_parameter("wqkT", [C, 2 * C], f32r, isOutput=False)
    wvT = nc.declare_dram_parameter("wvT", [C, C], f32r, isOutput=False)
    wpT = nc.declare_dram_parameter("wpT", [C, C], f32r, isOutput=False)
    cos2 = nc.declare_dram_parameter("cos2", [128, NS], f32, isOutput=False)
    sins2 = nc.declare_dram_parameter("sins2", [128, NS], f32, isOutput=False)
    biasb = nc.declare_dram_parameter("biasb", [128, C], f32, isOutput=False)
    out = nc.declare_dram_parameter("out", [NS, C], f32, isOutput=True)

    groups = [list(range(GB)), list(range(GB, 2 * GB))]

    def mm(out_ap, lhsT_ap, rhs_ap, start, stop):
        nc.tensor.matmul(out_ap, lhsT_ap, rhs_ap, start=start, stop=stop)

    from contextlib import ExitStack

    with tile.TileContext(nc) as tc:
        with ExitStack() as stack:
            ep = stack.enter_context
            ep(nc.allow_low_precision(reason="f32r rounding of fp32 matmul inputs"))
            dramp = ep(tc.tile_pool(name="dram", bufs=1, space="DRAM"))
            constp = ep(tc.tile_pool(name="const", bufs=1))
            xtp = ep(tc.tile_pool(name="xTp", bufs=1))
            qtp = ep(tc.tile_pool(name="qTp", bufs=1))
            atp = ep(tc.tile_pool(name="aTp", bufs=1))
            wtsp = ep(tc.tile_pool(name="wts", bufs=12))
            ktmpp = ep(tc.tile_pool(name="ktmp", bufs=3))
            ropep = ep(tc.tile_pool(name="ropet", bufs=3))
            kheadp = ep(tc.tile_pool(name="khead", bufs=2))
            ptp = ep(tc.tile_pool(name="pTp", bufs=3))
            vhp_p = ep(tc.tile_pool(name="vhp", bufs=4))
            smallp = ep(tc.tile_pool(name="small", bufs=4))
            outp = ep(tc.tile_pool(name="outsb", bufs=3))
            ps_mm = ep(tc.tile_pool(name="ps_mm", bufs=2, space="PSUM"))
            ps_s = ep(tc.tile_pool(name="ps_s", bufs=2, space="PSUM"))
            ps_av = ep(tc.tile_pool(name="ps_av", bufs=2, space="PSUM"))

            # ---- internal DRAM for collectives ----
            k_in = dramp.tile([C, NS], f32r, name="k_in")
            k_gath = dramp.tile([GB, C, NS], f32r, name="k_gath")
            v_in = dramp.tile([NS, H, HD + 1], f32r, name="v_in")
            v_gath = dramp.tile([GB, NS, H, HD + 1], f32r, name="v_gath")

            # ---- constants / persistent loads ----
            cos_sb = constp.tile([128, NS], f32, name="cos_sb")
            nc.sync.dma_start(cos_sb[:, :], cos2[:, :])
            sin_sb = constp.tile([128, NS], f32, name="sin_sb")
            nc.sync.dma_start(sin_sb[:, :], sins2[:, :])
            bias_sb = constp.tile([128, C], f32, name="bias_sb")
            nc.sync.dma_start(bias_sb[:, :], biasb[:, :])
            onesf = constp.tile([128, 64], f32, name="onesf")
            nc.vector.memset(onesf[:, :], 1.0)
            ones64 = constp.tile([1, 64], f32r, name="ones64")
            nc.vector.tensor_copy(ones64[:, :], onesf[0:1, :])

            xT_sb = xtp.tile([128, 8, NS], f32r, name="xT_sb")
            for cc in range(8):
                nc.sync.dma_start(
                    xT_sb[:, cc, :], xT[cc * 128:(cc + 1) * 128, :]
                )

            qT_sb = qtp.tile([128, 8, NS], f32r, name="qT_sb")
            aT_sb = atp.tile([128, 8, NS], f32r, name="aT_sb")

            def rope_chunk(psum, dst):
                """dst = psum*cos + rot32(psum)*signed_sin, all [128, NS]."""
                tmp = ropep.tile([128, NS], f32, name="tmp", tag="ropetmp")
                for lo in (0, 64):
                    nc.vector.tensor_mul(
                        tmp[lo:lo + 32, :],
                        psum[lo + 32:lo + 64, :],
                        sin_sb[lo:lo + 32, :],
                    )
                    nc.vector.tensor_mul(
                        tmp[lo + 32:lo + 64, :],
                        psum[lo:lo + 32, :],
                        sin_sb[lo + 32:lo + 64, :],
                    )
                nc.vector.tensor_mul(dst, psum, cos_sb[:, :])
                nc.vector.tensor_add(dst, dst, tmp[:, :])

            # ---- v natural layout [i, dv] -> v_in (ones col) -> AllGather ----
            wv_tiles = []
            for cc in range(8):
                w = wtsp.tile([128, 2, 512], f32r, name="w", tag="wts")
                nc.sync.dma_start(
                    w[:, :, :],
                    wvT[cc * 128:(cc + 1) * 128, :].rearrange(
                        "p (n f) -> p n f", f=512
                    ),
                )
                wv_tiles.append(w)
            for ic in range(4):
                rows = slice(ic * 128, (ic + 1) * 128)
                vsb = outp.tile([128, H, HD + 1], f32r, name="vsb", tag="osb")
                nc.vector.tensor_copy(vsb[:, :, HD], onesf[:, 0:H])
                for nn in range(2):
                    psum = ps_mm.tile([128, NS], f32, name="psum", tag="mm")
                    for cc in range(8):
                        mm(psum[:, :], xT_sb[:, cc, rows], wv_tiles[cc][:, nn, :],
                           cc == 0, cc == 7)
                    nc.vector.tensor_copy(
                        vsb[:, nn * 8:(nn + 1) * 8, 0:HD],
                        psum[:, :].rearrange("p (h d) -> p h d", d=HD),
                    )
                nc.scalar.dma_start(v_in[rows, :, :], vsb[:, :, :])
            if mock_ag:
                for r in range(GB):
                    nc.gpsimd.dma_start(
                        v_gath[r, 0:32, :, :], v_in[0:32, :, :]
                    )
            else:
                nc.gpsimd.collective_compute(
                    "AllGather",
                    mybir.AluOpType.bypass,
                    replica_groups=groups,
                    ins=[v_in.opt()],
                    outs=[v_gath.opt()],
                )

            # ---- k^T group (dq chunks 8..15) -> rope -> k_in -> AllGather ----
            wk_tiles = []
            for cc in range(8):
                w = wtsp.tile([128, 8, 128], f32r, name="w", tag="wts")
                nc.scalar.dma_start(
                    w[:, :, :],
                    wqkT[cc * 128:(cc + 1) * 128, C:2 * C].rearrange(
                        "p (m f) -> p m f", f=128
                    ),
                )
                wk_tiles.append(w)
            for m in range(8):
                psum = ps_mm.tile([128, NS], f32, name="psum", tag="mm")
                for cc in range(8):
                    mm(psum[:, :], wk_tiles[cc][:, m, :], xT_sb[:, cc, :],
                       cc == 0, cc == 7)
                kc = ktmpp.tile([128, NS], f32r, name="kc", tag="kc")
                rope_chunk(psum[:, :], kc[:, :])
                nc.scalar.dma_start(k_in[m * 128:(m + 1) * 128, :], kc[:, :])

            if mock_ag:
                # timing stand-in: ~real AllGather latency, correct dep edges
                for r in range(GB):
                    nc.gpsimd.dma_start(
                        k_gath[r, 0:64, :], k_in[0:64, :]
                    )
            else:
                nc.gpsimd.collective_compute(
                    "AllGather",
                    mybir.AluOpType.bypass,
                    replica_groups=groups,
                    ins=[k_in.opt()],
                    outs=[k_gath.opt()],
                )

            # ---- q^T group (dq chunks 0..7) + rope, overlaps the gathers ----
            wq_tiles = []
            for cc in range(8):
                w = wtsp.tile([128, 8, 128], f32r, name="w", tag="wts")
                nc.scalar.dma_start(
                    w[:, :, :],
                    wqkT[cc * 128:(cc + 1) * 128, 0:C].rearrange(
                        "p (m f) -> p m f", f=128
                    ),
                )
                wq_tiles.append(w)
            for m in range(8):
                psum = ps_mm.tile([128, NS], f32, name="psum", tag="mm")
                for cc in range(8):
                    mm(psum[:, :], wq_tiles[cc][:, m, :], xT_sb[:, cc, :],
                       cc == 0, cc == 7)
                rope_chunk(psum[:, :], qT_sb[:, m, :])

            # ---- attention, head pairs (flash-style over key chunks) ----
            vg = {}
            for hp in range(H // 2):  # heads 2*hp, 2*hp+1
                if hp % 4 == 0:  # prefetch v for heads [8*g, 8*(g+1))
                    g = hp // 4
                    for r in range(GB):
                        vt = vhp_p.tile(
                            [128, GB, 8, HD + 1], f32r, name="vt", tag="vt"
                        )
                        nc.gpsimd.dma_start(
                            vt[:, :, :, :],
                            v_gath[r, :, g * 8:(g + 1) * 8, :].rearrange(
                                "(a p) h d -> p a h d", p=128
                            ),
                        )
                        vg[r] = vt
                kh = kheadp.tile([128, GB, NS], f32r, name="kh", tag="khead")
                for r in range(GB):
                    nc.gpsimd.dma_start(
                        kh[:, r, :], k_gath[r, hp * 128:(hp + 1) * 128, :]
                    )
                for sub in range(2):  # head h = 2*hp + sub at partitions sub*64
                    h = 2 * hp + sub
                    lo = sub * 64
                    q_ap = qT_sb[lo:lo + 64, hp, :]
                    po = ps_av.tile([HD + 1, NS], f32, name="po", tag="av")
                    for jp in range(8):  # pairs of key chunks
                        jc0 = 2 * jp
                        ps2 = ps_s.tile([128, 2, NS], f32, name="ps2", tag="sc")
                        for u in range(2):
                            jc = jc0 + u
                            r, jl = jc // 4, jc % 4
                            mm(ps2[:, u, :],
                               kh[lo:lo + 64, r, jl * 128:(jl + 1) * 128],
                               q_ap, True, True)
                        pt = ptp.tile([128, 2, NS], f32r, name="pt", tag="pT")
                        nc.scalar.activation(
                            pt[:, :, :], ps2[:, :, :], AF.Exp, scale=SC
                        )
                        for u in range(2):
                            jc = jc0 + u
                            r, jl = jc // 4, jc % 4
                            mm(po[:, :],
                               vg[r][:, jl, 2 * (hp % 4) + sub, :],
                               pt[:, u, :], jc == 0, jc == 15)
                    # normalize: reciprocal of denom row, bcast via K=1 matmul
                    recip = smallp.tile([1, NS], f32r, name="recip", tag="recip")
                    nc.vector.reciprocal(recip[:, :], po[HD:HD + 1, :])
                    pb = ps_mm.tile([64, NS], f32, name="pb", tag="mm")
                    mm(pb[:, :], ones64[:, :], recip[:, :], True, True)
                    rb = smallp.tile([64, NS], f32, name="rb", tag="rb")
                    nc.vector.tensor_copy(rb[:, :], pb[:, :])
                    nc.vector.tensor_mul(
                        aT_sb[lo:lo + 64, hp, :], po[0:HD, :], rb[:, :]
                    )

            # ---- projection ----
            wp_tiles = []
            for cc in range(8):
                w = wtsp.tile([128, 2, 512], f32r, name="w", tag="wts")
                nc.sync.dma_start(
                    w[:, :, :],
                    wpT[cc * 128:(cc + 1) * 128, :].rearrange(
                        "p (n f) -> p n f", f=512
                    ),
                )
                wp_tiles.append(w)
            for ic in range(4):
                rows = slice(ic * 128, (ic + 1) * 128)
                for nn in range(2):
                    psum = ps_mm.tile([128, NS], f32, name="psum", tag="mm")
                    for cc in range(8):
                        mm(psum[:, :], aT_sb[:, cc, rows], wp_tiles[cc][:, nn, :],
                           cc == 0, cc == 7)
                    osb = outp.tile([128, 512], f32, name="osb", tag="osb")
                    nc.vector.tensor_add(
                        osb[:, :], psum[:, :], bias_sb[:, nn * 512:(nn + 1) * 512]
                    )
                    nc.sync.dma_start(out[rows, nn * 512:(nn + 1) * 512], osb[:, :])

    nc.compile()
    return nc


_NC_CACHE = {}


def _get_nc():
    if "nc" not in _NC_CACHE:
        _NC_CACHE["nc"] = build()
    return _NC_CACHE["nc"]


def make_in_maps(x, cos, sin, qkv_w, proj_w, proj_b):
    x = np.asarray(x, np.float32)
    cos = np.asarray(cos, np.float32)
    sin = np.asarray(sin, np.float32)
    qkv_w = np.asarray(qkv_w, np.float32)
    proj_w = np.asarray(proj_w, np.float32)
    proj_b = np.asarray(proj_b, np.float32)

    wqkT = np.ascontiguousarray(qkv_w[: 2 * C].T)        # [C, 2C]
    wvT = np.ascontiguousarray(qkv_w[2 * C:].T)          # [C, C]
    wpT = np.ascontiguousarray(proj_w.T)                 # [C, C]
    biasb = np.ascontiguousarray(np.broadcast_to(proj_b, (128, C)))
    sign = np.concatenate([-np.ones(32, np.float32), np.ones(32, np.float32)])

    in_maps = []
    for c in range(NCORES):
        b, r = c // GB, c % GB
        rows = slice(r * NS, (r + 1) * NS)
        xTc = np.ascontiguousarray(x[b, rows].T)          # [C, NS]
        cosT = cos[rows].T                                # [HD, NS]
        sinsT = (sin[rows] * sign).T                      # [HD, NS] signed
        cos2v = np.ascontiguousarray(np.concatenate([cosT, cosT], 0))   # [128, NS]
        sins2v = np.ascontiguousarray(np.concatenate([sinsT, sinsT], 0))
        in_maps.append(
            {
                "xT": xTc,
                "wqkT": wqkT,
                "wvT": wvT,
                "wpT": wpT,
                "cos2": cos2v,
                "sins2": sins2v,
                "biasb": biasb,
            }
        )
    return in_maps


def assemble(results):
    out = np.empty((B, N, C), np.float32)
    for c in range(NCORES):
        b, r = c // GB, c % GB
        out[b, r * NS:(r + 1) * NS] = results[c]["out"]
    return out


def kernel(x, cos, sin, qkv_w, proj_w, proj_b):
    from concourse.bass_utils import run_bass_kernel_spmd

    nc = _get_nc()
    in_maps = make_in_maps(x, cos, sin, qkv_w, proj_w, proj_b)
    res = run_bass_kernel_spmd(nc, in_maps, core_ids=list(range(NCORES)))
    return assemble(res.results)
